# revision 17
# baseline (speedup 1.0000x reference)
"""Trainium2 Bass kernel for nn_EmbedMatcher (GNN message passing).

Data-parallel over B=1024 across 8 cores (128 rows each); the 200001x128
symbol table is replicated per core, stored bf16 (the K-sum tolerates it).

Gather strategy (the memory-bound phase): the wide-gather ucode
(InstDMAGatherAnt) moves thousands of rows per instruction but takes int16
indices, so the host splits the table into 7 buckets of 28572 rows and
emits, per (half, bucket), a column-aligned index matrix: slot (m, b)
holds the m-th index of batch row b falling in that bucket (pad = bucket
base row, corrected by a host-computed correction tile). Gathered row
j=m*128+b lands on partition b, free block m, so a strided tensor_reduce
yields per-batch sums directly. Indices past Q=30 per (bucket, batch)
ride along as 2 densely-packed overflow blocks per bucket whose rows are
routed to their batch by one-hot PE matmuls (dead pads carry batch id 999
-> all-zero one-hot). Each (half, bucket) is split into two 2048-row
gather instructions spread over all 4 SWDGE queues: descriptor generation
on the Q7 cores (~7ns/descriptor/queue) is the bottleneck, not HBM
bandwidth, and 12 rotating buffers keep dispatch free of consumer-wait
stalls.

The GCN linear is algebraically reordered: sum_k (concat @ W^T) ==
[rel_sum|ent_sum] @ W^T. LSTM step-0 gates depend only on the query row
(whose f32 embedding the host supplies directly) and are computed during
the gather phase. [support_g | support_g^T] is AllGathered so each core
runs the batch-coupled attention on its own 128 query rows without
post-collective transposes. Sigmoids are 0.5*tanh(x/2)+0.5 so the ACT
engine stays inside one function-table set (exp/tanh) across the loop.
"""
import numpy as np

from concourse import bass, bacc, mybir
import concourse.tile as tile
from concourse.bass_utils import run_bass_kernel_spmd

P = 128            # batch rows per core
D = 128            # embed dim
K = 200            # neighbors
NCORES = 8
NROWS = 200001     # symbol table rows (incl. zero padding row 200000)
STEPS = 4
NBUK = 7           # int16 table buckets
BW = 28572         # bucket width (7*28572 >= 200001, < 32768)
Q = 30             # column slots per (bucket, batch)
OVB = 2            # overflow blocks (128 rows each) per (half, bucket)
NBLK = Q + OVB     # 32 gathered blocks per (half, bucket) pair
NBUF = 12
F32 = mybir.dt.float32
BF16 = mybir.dt.bfloat16
I16 = mybir.dt.int16

# pair p = (half, bucket); pairs 0-11 gather as one 4096-row instruction,
# pairs 12/13 split into 2048-row chunks so every SWDGE queue carries the
# same generation load (3 full + 1 half = ~111us each)
PAIRS = [(0, 0), (0, 1), (0, 2), (1, 0), (1, 1), (1, 2),
         (0, 3), (0, 4), (0, 5), (1, 3), (1, 4), (1, 5),
         (0, 6), (1, 6)]
# gather instruction list: (pair, chunk); chunk None = full pair
GLIST = [(p, None) for p in range(12)] + [(12, 0), (12, 1), (13, 0), (13, 1)]
NGATH = len(GLIST)
# stripe columns per instruction and cumulative offsets
_GCOLS = [(P * NBLK if ch is None else P * NBLK // 2) // 16 for _, ch in GLIST]
_GOFF = [0]
for _n_ in _GCOLS:
    _GOFF.append(_GOFF[-1] + _n_)
BIDX_COLS = _GOFF[-1]

_SIM_SINGLE_QUEUE = False

_CACHE = {}

# weight pack layout: name -> (col_start, ncols), all [128, n] f32 blocks
_WSPECS = [
    ("wrT", D), ("weT", D), ("gcnb", D),
    ("p1wT", 2 * D), ("p1b", 2),
    ("p2wTa", D), ("p2wTb", D), ("p2b", D),
    ("lna", D), ("lnb", D),
    ("wihT", 8 * D), ("whhTa", 8 * D), ("whhTb", 8 * D), ("gbias", 8 * D),
    ("corr_rel", D), ("corr_ent", D), ("iota", D), ("ovb", 14 * OVB),
    ("qrows", D), ("ident", P),
]
_WOFF = {}
_c = 0
for _n, _w in _WSPECS:
    _WOFF[_n] = (_c, _w)
    _c += _w
WCOLS = _c


def _build():
    nc = bacc.Bacc("TRN2", target_bir_lowering=False, debug=False,
                   enable_asserts=True, num_devices=NCORES,
                   num_swdge_queues=4)
    ap = {}
    ap["table"] = nc.dram_tensor("table", [NROWS, D], BF16,
                                 kind="ExternalInput").ap()
    ap["bidx"] = nc.dram_tensor("bidx", [P, BIDX_COLS], I16,
                                kind="ExternalInput").ap()
    ap["wpack"] = nc.dram_tensor("wpack", [P, WCOLS], F32,
                                 kind="ExternalInput").ap()
    out_d = nc.dram_tensor("out", [P, 1], F32, kind="ExternalOutput").ap()

    AX = mybir.AxisListType.X
    OP = mybir.AluOpType
    ACT = mybir.ActivationFunctionType

    with tile.TileContext(nc, num_cores=NCORES) as tc:
        with tc.tile_pool(name="sb", bufs=1) as sb, \
             tc.tile_pool(name="ps", bufs=2, space="PSUM") as ps, \
             tc.tile_pool(name="pst", bufs=2, space="PSUM") as pst, \
             tc.tile_pool(name="dram", bufs=1, space="DRAM") as dram:

            # ---- load inputs to SBUF
            bidx_sb = sb.tile([P, BIDX_COLS], I16)
            nc.sync.dma_start(out=bidx_sb[:], in_=ap["bidx"][:])
            wsb = sb.tile([P, WCOLS], F32)
            nc.sync.dma_start(out=wsb[:], in_=ap["wpack"][:])

            def w(name):
                c0, n = _WOFF[name]
                return wsb[:, c0:c0 + n]

            # ---- bucket gathers; overflow rows routed by one-hot matmuls
            gbuf = [sb.tile([P, NBLK * D], BF16, name=f"gbuf{i}")
                    for i in range(NBUF)]
            ov_ps = [ps.tile([P, D], F32, name=f"ovps{h}", tag="mm")
                     for h in range(2)]
            ov_mm = [0, 0]

            def bucket_gather(i):
                buf = gbuf[i % NBUF]
                pair, ch = GLIST[i]
                base = PAIRS[pair][1] * BW
                nidx = _GCOLS[i] * 16
                nc.gpsimd.dma_gather(
                    out_ap=buf[:, :nidx * D // P].rearrange(
                        "p (m d) -> p m d", d=D),
                    in_ap=ap["table"][base:min(base + BW, NROWS)],
                    idxs_ap=bidx_sb[:, _GOFF[i]:_GOFF[i + 1]],
                    num_idxs=nidx, num_idxs_reg=nidx, elem_size=D,
                    single_packet=False,
                    queue_num=0 if _SIM_SINGLE_QUEUE else i % 4)
                return buf

            wave1 = [bucket_gather(i) for i in range(NBUF)]
            # (reduces for wave-1 are issued after the step-0 precompute so
            # the precompute's DVE ops aren't queued behind gather waits)

            def transpose_to(dst_sb, src_ap, nm):
                tp = pst.tile([P, P], F32, name=f"tp_{nm}", tag="tp")
                nc.tensor.transpose(out=tp[:], in_=src_ap,
                                    identity=w("ident"))
                nc.vector.tensor_copy(out=dst_sb, in_=tp[:])

            # ---- LSTM step-0 precompute (query row + weights only)
            qT = sb.tile([P, P], F32)
            transpose_to(qT[:], w("qrows"), "q")
            gts_q = sb.tile([P, 8 * D], F32)   # q @ w_ih.T + b_ih + b_hh
            for j in range(2):
                gp = ps.tile([P, 512], F32, name=f"gq{j}", tag="gates")
                sl = slice(512 * j, 512 * (j + 1))
                nc.tensor.matmul(out=gp[:], lhsT=qT[:], rhs=w("wihT")[:, sl],
                                 start=True, stop=True)
                nc.vector.tensor_add(out=gts_q[:, sl], in0=gp[:],
                                     in1=w("gbias")[:, sl])
            c_sb = sb.tile([P, 2 * D], F32)
            si = sb.tile([P, 2 * D], F32)
            sf = sb.tile([P, 2 * D], F32)
            tg = sb.tile([P, 2 * D], F32)
            so = sb.tile([P, D], F32)
            tch = sb.tile([P, D], F32)
            ho = sb.tile([P, D], F32)
            hoT = sb.tile([P, P], F32)

            def sigmoid(dst, src_ap):
                # 0.5*tanh(x/2)+0.5 keeps ACT inside the exp/tanh table set
                # (no 1.3us table reloads) and avoids slow DVE reciprocals
                nc.scalar.activation(out=dst, in_=src_ap, func=ACT.Tanh,
                                     scale=0.5)
                nc.vector.tensor_scalar(out=dst, in0=dst, scalar1=0.5,
                                        scalar2=0.5, op0=OP.mult, op1=OP.add)

            sigmoid(si[:], gts_q[:, 0:256])
            nc.scalar.activation(out=tg[:], in_=gts_q[:, 512:768], func=ACT.Tanh)
            sigmoid(so[:], gts_q[:, 768:896])
            nc.vector.tensor_tensor(out=c_sb[:], in0=si[:], in1=tg[:], op=OP.mult)
            nc.scalar.activation(out=tch[:], in_=c_sb[:, 0:D], func=ACT.Tanh)
            nc.vector.tensor_tensor(out=tch[:], in0=so[:], in1=tch[:], op=OP.mult)
            nc.vector.tensor_add(out=ho[:], in0=w("qrows"), in1=tch[:])
            transpose_to(hoT[:], ho[:], "ho0")

            # ---- reduce each gather chunk and accumulate (interleaved with
            # the remaining gathers so buffers are consumed before reuse)
            acc = [sb.tile([P, D], F32, name=f"acc{h}") for h in range(2)]
            first = [True, True]
            tmp = [sb.tile([P, D], F32, name=f"rtmp{i}") for i in range(2)]
            htmp = [sb.tile([P, (Q // 2) * D], F32, name=f"htmp{i}")
                    for i in range(2)]

            def reduce_and_acc(i):
                pair, ch = GLIST[i]
                h = PAIRS[pair][0]
                buf = gbuf[i % NBUF]
                # aligned blocks first, then OVB overflow blocks (full pair
                # and chunk-1 instructions carry the pair's overflow)
                nali = Q if ch is None else (16 if ch == 0 else 16 - OVB)
                # contiguous halving add (cheap) then a half-length strided
                # reduce: ~2x cheaper than one full-depth strided reduce
                nh = nali // 2
                hv = htmp[i % 2]
                nc.vector.tensor_tensor(out=hv[:, :nh * D],
                                        in0=buf[:, :nh * D],
                                        in1=buf[:, nh * D:2 * nh * D],
                                        op=OP.add)
                view = hv[:, :nh * D].rearrange("p (m d) -> p d m", d=D)
                if first[h]:
                    nc.vector.tensor_reduce(out=acc[h][:], in_=view,
                                            axis=AX, op=OP.add)
                    first[h] = False
                else:
                    t = tmp[i % 2]
                    nc.vector.tensor_reduce(out=t[:], in_=view,
                                            axis=AX, op=OP.add)
                    nc.vector.tensor_add(out=acc[h][:], in0=acc[h][:],
                                         in1=t[:])
                if ch == 0:
                    return
                for j in range(OVB):
                    col = pair * OVB + j
                    oh = sb.tile([P, P], BF16, name=f"oh{i}{j}", tag="oh",
                                 bufs=32)
                    nc.vector.tensor_scalar(
                        out=oh[:], in0=w("iota"),
                        scalar1=w("ovb")[:, col:col + 1], scalar2=None,
                        op0=OP.is_equal)
                    nc.tensor.matmul(
                        out=ov_ps[h][:], lhsT=oh[:],
                        rhs=buf[:, (nali + j) * D:(nali + j + 1) * D],
                        start=(ov_mm[h] == 0),
                        stop=(ov_mm[h] == NBUK * OVB - 1))
                    ov_mm[h] += 1

            for i in range(NBUF, NGATH):
                reduce_and_acc(i - NBUF)
                bucket_gather(i)
            for i in range(NGATH - NBUF, NGATH):
                reduce_and_acc(i)
            # pad correction + overflow contribution
            rel_sum = sb.tile([P, D], F32)
            ent_sum = sb.tile([P, D], F32)
            nc.vector.tensor_tensor(out=rel_sum[:], in0=acc[0][:],
                                    in1=w("corr_rel"), op=OP.subtract)
            nc.vector.tensor_add(out=rel_sum[:], in0=rel_sum[:],
                                 in1=ov_ps[0][:])
            nc.vector.tensor_tensor(out=ent_sum[:], in0=acc[1][:],
                                    in1=w("corr_ent"), op=OP.subtract)
            nc.vector.tensor_add(out=ent_sum[:], in0=ent_sum[:],
                                 in1=ov_ps[1][:])

            # ---- GCN: support = tanh((rel_sum@Wr' + ent_sum@We') + gcnb)
            relT = sb.tile([P, P], F32)
            transpose_to(relT[:], rel_sum[:], "rel")
            entT = sb.tile([P, P], F32)
            transpose_to(entT[:], ent_sum[:], "ent")
            sup_ps = ps.tile([P, D], F32, name="sup_ps", tag="mm")
            nc.tensor.matmul(out=sup_ps[:], lhsT=relT[:], rhs=w("wrT"),
                             start=True, stop=False)
            nc.tensor.matmul(out=sup_ps[:], lhsT=entT[:], rhs=w("weT"),
                             start=False, stop=True)
            support = sb.tile([P, D], F32)
            nc.vector.tensor_add(out=support[:], in0=sup_ps[:], in1=w("gcnb"))
            nc.scalar.activation(out=support[:], in_=support[:], func=ACT.Tanh)

            # ---- FFN + residual + layernorm -> support_g
            supT = sb.tile([P, P], F32)
            transpose_to(supT[:], support[:], "sup")
            hidT = []
            for j in range(2):
                hp = ps.tile([P, D], F32, name=f"hid_ps{j}", tag="mm")
                nc.tensor.matmul(out=hp[:], lhsT=w("p1wT")[:, j * D:(j + 1) * D],
                                 rhs=supT[:], start=True, stop=True)
                ht = sb.tile([P, P], F32, name=f"hidT{j}")
                nc.scalar.activation(out=ht[:], in_=hp[:], func=ACT.Relu,
                                     bias=w("p1b")[:, j:j + 1])
                hidT.append(ht)
            o2 = ps.tile([P, D], F32, name="o2", tag="mm")
            nc.tensor.matmul(out=o2[:], lhsT=hidT[0][:], rhs=w("p2wTa"),
                             start=True, stop=False)
            nc.tensor.matmul(out=o2[:], lhsT=hidT[1][:], rhs=w("p2wTb"),
                             start=False, stop=True)
            z = sb.tile([P, D], F32)
            nc.vector.tensor_add(out=z[:], in0=o2[:], in1=support[:])
            nc.vector.tensor_add(out=z[:], in0=z[:], in1=w("p2b"))
            # layernorm (unbiased std, eps added to std)
            zsum = sb.tile([P, 1], F32)
            nc.vector.tensor_reduce(out=zsum[:], in_=z[:], axis=AX, op=OP.add)
            zmean = sb.tile([P, 1], F32)
            nc.scalar.mul(out=zmean[:], in_=zsum[:], mul=1.0 / D)
            xc = sb.tile([P, D], F32)
            nc.vector.tensor_scalar(out=xc[:], in0=z[:], scalar1=zmean[:, 0:1],
                                    scalar2=None, op0=OP.subtract)
            sqt = sb.tile([P, D], F32)
            varsum = sb.tile([P, 1], F32)
            nc.scalar.activation(out=sqt[:], in_=xc[:], func=ACT.Square,
                                 accum_out=varsum[:])
            sigma = sb.tile([P, 1], F32)
            nc.scalar.activation(out=sigma[:], in_=varsum[:], func=ACT.Sqrt,
                                 scale=1.0 / (D - 1))
            nc.vector.tensor_scalar(out=sigma[:], in0=sigma[:], scalar1=1e-3,
                                    scalar2=None, op0=OP.add)
            rec = sb.tile([P, 1], F32)
            nc.vector.reciprocal(out=rec[:], in_=sigma[:])
            sg = sb.tile([P, D], F32)
            nc.vector.tensor_scalar(out=sg[:], in0=xc[:], scalar1=rec[:, 0:1],
                                    scalar2=None, op0=OP.mult)
            nc.vector.tensor_tensor(out=sg[:], in0=sg[:], in1=w("lna"),
                                    op=OP.mult)
            nc.vector.tensor_tensor(out=sg[:], in0=sg[:], in1=w("lnb"),
                                    op=OP.add)

            # ---- AllGather [support_g | support_g^T]: both layouts arrive
            # without 8 post-collective transposes
            sgT_own = sb.tile([P, P], F32)
            transpose_to(sgT_own[:], sg[:], "sgt")
            ag_in = dram.tile([P, 2 * D], F32)
            ag_out = dram.tile([NCORES * P, 2 * D], F32)
            nc.sync.dma_start(out=ag_in[:, 0:D], in_=sg[:])
            nc.sync.dma_start(out=ag_in[:, D:2 * D], in_=sgT_own[:])
            nc.gpsimd.collective_compute(
                "AllGather", OP.bypass,
                replica_groups=[list(range(NCORES))],
                ins=[ag_in.opt()], outs=[ag_out.opt()])
            sg_all = sb.tile([P, NCORES, D], F32)
            nc.sync.dma_start(
                out=sg_all[:],
                in_=ag_out[:, 0:D].rearrange("(c p) d -> p c d", c=NCORES))
            sgT = sb.tile([P, NCORES * P], F32)
            nc.sync.dma_start(
                out=sgT[:].rearrange("p (c b) -> p c b", c=NCORES),
                in_=ag_out[:, D:2 * D].rearrange("(c p) b -> p c b",
                                                 c=NCORES))

            # ---- LSTM + attention (step-0 state precomputed above)
            gts = sb.tile([P, 8 * D], F32)
            rT_sb = sb.tile([P, P], F32)
            attn = sb.tile([P, NCORES * P], F32)
            rowsum = sb.tile([P, 1], F32)
            rsrec = sb.tile([P, 1], F32)

            for s in range(STEPS - 1):
                sc = ps.tile([P, NCORES * P], F32, name=f"sc{s}", tag="scores",
                             bufs=1)
                for j in range(2):
                    nc.tensor.matmul(out=sc[:, 512 * j:512 * (j + 1)],
                                     lhsT=hoT[:],
                                     rhs=sgT[:, 512 * j:512 * (j + 1)],
                                     start=True, stop=True)
                # softmax; exp without max-subtraction (|scores| <~ 60)
                nc.scalar.activation(out=attn[:], in_=sc[:], func=ACT.Exp,
                                     accum_out=rowsum[:])
                nc.vector.reciprocal(out=rsrec[:], in_=rowsum[:])
                nc.vector.tensor_scalar(out=attn[:], in0=attn[:],
                                        scalar1=rsrec[:, 0:1], scalar2=None,
                                        op0=OP.mult)
                rp = ps.tile([P, D], F32, name=f"rp{s}", tag="mm")
                for c in range(NCORES):
                    at = sb.tile([P, P], F32, name=f"at{s}{c}", tag="atT",
                                 bufs=2)
                    transpose_to(at[:], attn[:, c * P:(c + 1) * P], f"at{s}{c}")
                    nc.tensor.matmul(out=rp[:], lhsT=sg_all[:, c, :], rhs=at[:],
                                     start=(c == 0), stop=(c == NCORES - 1))
                nc.vector.tensor_copy(out=rT_sb[:], in_=rp[:])
                # gates for step s+1: gts_q + ho@WhhA' + r@WhhB'
                for j in range(2):
                    gp = ps.tile([P, 512], F32, name=f"g{s}{j}", tag="gates")
                    sl = slice(512 * j, 512 * (j + 1))
                    nc.tensor.matmul(out=gp[:], lhsT=hoT[:],
                                     rhs=w("whhTa")[:, sl],
                                     start=True, stop=False)
                    nc.tensor.matmul(out=gp[:], lhsT=rT_sb[:],
                                     rhs=w("whhTb")[:, sl],
                                     start=False, stop=True)
                    nc.vector.tensor_add(out=gts[:, sl], in0=gp[:],
                                         in1=gts_q[:, sl])
                sigmoid(si[:], gts[:, 0:256])
                nc.scalar.activation(out=tg[:], in_=gts[:, 512:768],
                                     func=ACT.Tanh)
                sigmoid(so[:], gts[:, 768:896])
                sigmoid(sf[:], gts[:, 256:512])
                nc.vector.tensor_tensor(out=sf[:], in0=sf[:], in1=c_sb[:],
                                        op=OP.mult)
                nc.vector.tensor_tensor(out=si[:], in0=si[:], in1=tg[:],
                                        op=OP.mult)
                nc.vector.tensor_add(out=c_sb[:], in0=sf[:], in1=si[:])
                nc.scalar.activation(out=tch[:], in_=c_sb[:, 0:D], func=ACT.Tanh)
                nc.vector.tensor_tensor(out=tch[:], in0=so[:], in1=tch[:],
                                        op=OP.mult)
                nc.vector.tensor_add(out=ho[:], in0=w("qrows"), in1=tch[:])
                if s < STEPS - 2:
                    transpose_to(hoT[:], ho[:], f"ho{s + 1}")

            # ---- cosine similarity against own support_g shard
            m1 = sb.tile([P, D], F32)
            nc.vector.tensor_tensor(out=m1[:], in0=ho[:], in1=sg[:], op=OP.mult)
            cross = sb.tile([P, 1], F32)
            nc.vector.tensor_reduce(out=cross[:], in_=m1[:], axis=AX, op=OP.add)
            n1 = sb.tile([P, 1], F32)
            n2 = sb.tile([P, 1], F32)
            nc.scalar.activation(out=m1[:], in_=ho[:], func=ACT.Square,
                                 accum_out=n1[:])
            nc.scalar.activation(out=m1[:], in_=sg[:], func=ACT.Square,
                                 accum_out=n2[:])
            nc.vector.tensor_tensor(out=n1[:], in0=n1[:], in1=n2[:], op=OP.mult)
            nc.scalar.activation(out=n1[:], in_=n1[:], func=ACT.Sqrt)
            nc.vector.reciprocal(out=n1[:], in_=n1[:])
            res = sb.tile([P, 1], F32)
            nc.vector.tensor_tensor(out=res[:], in0=cross[:], in1=n1[:],
                                    op=OP.mult)
            nc.sync.dma_start(out=out_d[:], in_=res[:])
    nc.compile()
    return nc


def _pack_stripes(jlist):
    """[NIDX] ints -> [128, NIDX//16] int16 tile: index j at [j%16, j//16],
    replicated down the 8 16-partition stripes (one per Q7 core)."""
    blk = jlist.astype(np.int16).reshape(-1, 16).T
    return np.tile(blk, (8, 1))


def _bucket_pack(idxm):
    """idxm [128, K] global indices -> per-bucket (stripe chunk pairs,
    ovb columns [128, OVB] f32) plus padcnt [128, 7].

    Each (half, bucket) emits 2 gather chunks of 2048 rows: chunk 0 =
    column-aligned slots m 0..15, chunk 1 = slots m 16..29 plus OVB*128
    densely packed overflow slots. Column-aligned pads gather the bucket
    base row (corrected via padcnt); overflow pads carry batch id 999 so
    their one-hot column is all-zero.
    """
    c = np.minimum(idxm // BW, NBUK - 1)
    loc = idxm - c * BW
    chunks, ovbs = [], []
    padcnt = np.zeros((P, NBUK), np.int64)
    for cc in range(NBUK):
        m = c == cc
        cnt = m.sum(axis=1)
        padcnt[:, cc] = np.maximum(Q - cnt, 0)
        colmat = np.zeros((P, Q), np.int64)   # pad = bucket base row
        rank = np.cumsum(m, axis=1) - 1
        bsel, ksel = np.nonzero(m)
        r = rank[bsel, ksel]
        take = r < Q
        colmat[bsel[take], r[take]] = loc[bsel[take], ksel[take]]
        spill = ~take
        ov_loc = loc[bsel[spill], ksel[spill]]
        ov_bat = bsel[spill]
        n = len(ov_loc)
        assert n <= OVB * P, f"bucket overflow {n} exceeds {OVB * P}"
        ovlist = np.zeros(OVB * P, np.int64)
        ovbat = np.full(OVB * P, 999, np.int64)
        ovlist[:n] = ov_loc
        ovbat[:n] = ov_bat
        chunks.append((colmat, ovlist))
        ovbs.append(ovbat.reshape(OVB, P).T.astype(np.float32))
    return chunks, ovbs, padcnt


def _prep_inputs(relations, entities, query, symbol_emb, gcn_w_w, gcn_w_b,
                 p1_w, p1_b, p2_w, p2_b, ln_a, ln_b, w_ih, w_hh, b_ih, b_hh):
    import ml_dtypes
    f32 = np.float32
    table_f32 = np.asarray(symbol_emb, f32)
    table = np.ascontiguousarray(table_f32.astype(ml_dtypes.bfloat16))
    B = relations.shape[0]
    rel = np.asarray(relations).astype(np.int64)
    ent = np.asarray(entities).astype(np.int64)
    qry = np.asarray(query).astype(np.int64)
    inv = f32(1.0 / B)                     # reference divides by B (quirk)
    wcommon = np.empty((P, _WOFF["corr_rel"][0]), f32)

    def put(buf, name, arr):
        c0, n = _WOFF[name]
        buf[:, c0:c0 + n] = arr

    p2wT = np.asarray(p2_w).T.astype(f32)
    whhT = np.asarray(w_hh).T.astype(f32)
    put(wcommon, "wrT", (np.asarray(gcn_w_w)[:, :D] * inv).T)
    put(wcommon, "weT", (np.asarray(gcn_w_w)[:, D:] * inv).T)
    put(wcommon, "gcnb", np.broadcast_to(np.asarray(gcn_w_b) * (K / B), (P, D)))
    put(wcommon, "p1wT", np.asarray(p1_w).T)
    put(wcommon, "p1b", np.asarray(p1_b).reshape(2, P).T)
    put(wcommon, "p2wTa", p2wT[:D])
    put(wcommon, "p2wTb", p2wT[D:])
    put(wcommon, "p2b", np.broadcast_to(np.asarray(p2_b), (P, D)))
    put(wcommon, "lna", np.broadcast_to(np.asarray(ln_a), (P, D)))
    put(wcommon, "lnb", np.broadcast_to(np.asarray(ln_b), (P, D)))
    put(wcommon, "wihT", np.asarray(w_ih).T)
    put(wcommon, "whhTa", whhT[:D])
    put(wcommon, "whhTb", whhT[D:])
    put(wcommon, "gbias", np.broadcast_to(np.asarray(b_ih) + np.asarray(b_hh),
                                          (P, 8 * D)))
    # the device sums bf16 rows; corrections must match the bf16 values
    bases = table[np.arange(NBUK) * BW].astype(f32)
    iota = np.broadcast_to(np.arange(P, dtype=f32), (P, P))
    ident = np.eye(P, dtype=f32)

    in_maps = []
    for core in range(NCORES):
        rows = slice(core * P, (core + 1) * P)
        halves, ov_halves = {}, {}
        corr = np.zeros((2, P, D), f32)
        for h, idx in enumerate((rel[rows], ent[rows])):
            chunks, ovbs, padcnt = _bucket_pack(idx)
            corr[h] = padcnt.astype(f32) @ bases
            halves[h] = chunks
            ov_halves[h] = ovbs
        blocks, ovcols = [], []
        for pair, ch in GLIST:
            h, c = PAIRS[pair]
            colmat, ovlist = halves[h][c]
            if ch is None:
                jl = np.concatenate([colmat.T.ravel(), ovlist])
            elif ch == 0:
                jl = colmat[:, :16].T.ravel()
            else:
                jl = np.concatenate([colmat[:, 16:].T.ravel(), ovlist])
            blocks.append(_pack_stripes(jl))
        for pair in range(len(PAIRS)):
            h, c = PAIRS[pair]
            ovcols.append(ov_halves[h][c])
        wpack = np.empty((P, WCOLS), f32)
        wpack[:, :wcommon.shape[1]] = wcommon
        put(wpack, "corr_rel", corr[0])
        put(wpack, "corr_ent", corr[1])
        put(wpack, "iota", iota)
        put(wpack, "ovb", np.concatenate(ovcols, axis=1))
        put(wpack, "qrows", table_f32[qry[rows]])
        put(wpack, "ident", ident)
        m = {
            "table": table,
            "bidx": np.ascontiguousarray(np.concatenate(blocks, axis=1)),
            "wpack": wpack,
        }
        in_maps.append(m)
    return in_maps


def kernel(**inputs) -> np.ndarray:
    if "nc" not in _CACHE:
        _CACHE["nc"] = _build()
    nc = _CACHE["nc"]
    in_maps = _prep_inputs(**inputs)
    res = run_bass_kernel_spmd(nc, in_maps, list(range(NCORES)), trace=False)
    return np.concatenate([res.results[c]["out"][:, 0] for c in range(NCORES)])
